# revision 6
# baseline (speedup 1.0000x reference)
"""Chamfer loss (two 16384x16384 1-NN searches + gathered MSE) on 8 Trainium2 cores.

Strategy (per core, queries sharded 8-way):
  - PE: score matrix S[i,j] = q_i . r_j - |r_j|^2/2 - |q_i|^2/2 = -d(i,j)/2 via
    K=5 augmented fp32r matmuls ([5,128] x [5,512] -> PSUM), 32 j-tiles per
    128-query block. argmax_j S = argmin_j dist.
  - DVE: pairwise-max fold of PSUM quads into fp16 (Spair 8192 -> Squad 4096),
    then max/max_index top-8 slots -> 32 original candidate indices per query.
  - Exact refinement: indirect-DMA gather of the 32 candidate rows (fp32
    coords + norm), recompute exact fp32 scores, first-index argmax ->
    exact fp32 1-NN index (immune to fp32r/fp16 approximation noise).
  - MSE: indirect-DMA gather of ref-side e rows by final index, squared-diff
    accumulated per partition; host sums 128x2x8 partials and scales.

The kernel is self-contained: hardcoded shapes for V=16384, 8 cores.
"""
import sys

sys.path.insert(0, "/opt/trn_rl_repo")

import numpy as np

import concourse.bass as bass
import concourse.bacc as bacc
import concourse.mybir as mybir
from concourse.tile import TileContext
from concourse.bass_utils import run_bass_kernel_spmd

P = 128          # partitions / queries per block
V = 16384        # reference points
NCORES = 8
QPC = V // NCORES            # queries per core per direction (2048)
NBLK = QPC // P              # query blocks per direction (16)
NT = V // 512                # j-tiles of 512 (32)
NQUAD = NT // 4              # psum quads of 4 tiles (8)
NCAND = 32                   # candidates per query after top-8 quad slots
ACT_QUADS = 6                # PSUM->SBUF copy quads handled by ScalarE (rest: DVE)
F16 = mybir.dt.float16
F32 = mybir.dt.float32
F32R = mybir.dt.float32r
I32 = mybir.dt.int32
U32 = mybir.dt.uint32

_CACHE = {}


def build(n_blocks=NBLK):
    nc = bacc.Bacc()
    # per-core inputs
    qT = nc.dram_tensor("qT", [5, 2 * QPC], F32, kind="ExternalInput")
    rT = nc.dram_tensor("rT", [5, 2 * V], F32, kind="ExternalInput")
    rrowA = nc.dram_tensor("rrowA", [V, 4], F32, kind="ExternalInput")
    rrowB = nc.dram_tensor("rrowB", [V, 4], F32, kind="ExternalInput")
    erowA = nc.dram_tensor("erowA", [V, 4], F32, kind="ExternalInput")
    erowB = nc.dram_tensor("erowB", [V, 4], F32, kind="ExternalInput")
    qrow = nc.dram_tensor("qrow", [2 * QPC, 4], F32, kind="ExternalInput")
    eq = nc.dram_tensor("eq", [2 * QPC, 4], F32, kind="ExternalInput")
    loss_out = nc.dram_tensor("loss_out", [P, 2], F32, kind="ExternalOutput")
    idx_out = nc.dram_tensor("idx_out", [P, 2 * n_blocks], I32, kind="ExternalOutput")

    rrow_d = [rrowA, rrowB]
    erow_d = [erowA, erowB]

    with TileContext(nc) as tc:
        with (
            tc.tile_pool(name="tab", bufs=1) as tab,
            tc.tile_pool(name="sb", bufs=2) as sb,
            tc.tile_pool(name="fold", bufs=1) as fold,
            tc.tile_pool(name="small", bufs=3) as sm,
            tc.tile_pool(name="acc", bufs=1) as accp,
            tc.tile_pool(name="ps", bufs=2, space="PSUM") as ps,
        ):
            qt = tab.tile([5, 2 * QPC], F32R)
            nc.sync.dma_start(out=qt[:], in_=qT[:].bitcast(F32R))
            lossparts = accp.tile([P, 2 * n_blocks], F32)
            nc.vector.memset(lossparts[:], 0.0)
            idxbuf = accp.tile([P, 2 * n_blocks], I32)

            for d in range(2):
                rt = tab.tile([5, V], F32R, tag="rt")
                nc.sync.dma_start(
                    out=rt[:], in_=rT[:, d * V : (d + 1) * V].bitcast(F32R)
                )
                for b in range(n_blocks):
                    lhsT = qt[:, d * QPC + b * P : d * QPC + (b + 1) * P]
                    sfull = sb.tile([P, V], F16, tag="sfull")
                    for g in range(NQUAD):
                        psq = ps.tile([P, 2048], F32, tag="psq")
                        for t in range(4):
                            nc.tensor.matmul(
                                out=psq[:, t * 512 : (t + 1) * 512],
                                lhsT=lhsT,
                                rhs=rt[:, (g * 4 + t) * 512 : (g * 4 + t + 1) * 512],
                                start=True,
                                stop=True,
                            )
                        dst = sfull[:, g * 2048 : (g + 1) * 2048]
                        if g < ACT_QUADS:
                            nc.scalar.copy(dst, psq[:])
                        else:
                            nc.vector.tensor_copy(dst, psq[:])
                    spair = fold.tile([P, 8192], F16, tag="spair")
                    nc.vector.tensor_tensor(
                        out=spair[:],
                        in0=sfull[:, 0:8192],
                        in1=sfull[:, 8192:V],
                        op=mybir.AluOpType.max,
                    )
                    squad = fold.tile([P, 4096], F16, tag="squad")
                    nc.vector.tensor_tensor(
                        out=squad[:],
                        in0=spair[:, 0:4096],
                        in1=spair[:, 4096:8192],
                        op=mybir.AluOpType.max,
                    )
                    m8 = sm.tile([P, 8], F16, tag="m8")
                    i8 = sm.tile([P, 8], U32, tag="i8")
                    nc.vector.max(out=m8[:], in_=squad[:])
                    nc.vector.max_index(out=i8[:], in_max=m8[:], in_values=squad[:])

                    # slot v -> candidates v + {0, 4096, 8192, 12288}
                    i8f = sm.tile([P, 8], F32, tag="i8f")
                    nc.vector.tensor_copy(i8f[:], i8[:])
                    canf = sm.tile([P, NCAND], F32, tag="canf")
                    for o, off in enumerate([0.0, 4096.0, 8192.0, 12288.0]):
                        nc.vector.tensor_scalar(
                            out=canf[:, o * 8 : (o + 1) * 8], in0=i8f[:],
                            scalar1=off, scalar2=None, op0=mybir.AluOpType.add,
                        )
                    cani = sm.tile([P, NCAND], I32, tag="cani")
                    nc.vector.tensor_copy(cani[:], canf[:])

                    # gather candidate rows (x, y, z, -|r|^2/2) and refine exactly
                    rg = sm.tile([P, NCAND * 4], F32, tag="rg")
                    for k in range(NCAND):
                        nc.gpsimd.indirect_dma_start(
                            out=rg[:, k * 4 : (k + 1) * 4],
                            out_offset=None,
                            in_=rrow_d[d][:],
                            in_offset=bass.IndirectOffsetOnAxis(
                                ap=cani[:, k : k + 1], axis=0
                            ),
                        )
                    qr = sm.tile([P, 4], F32, tag="qr")
                    nc.sync.dma_start(
                        out=qr[:], in_=qrow[d * QPC + b * P : d * QPC + (b + 1) * P, :]
                    )
                    prod = sm.tile([P, NCAND * 4], F32, tag="prod")
                    nc.vector.tensor_tensor(
                        out=prod[:].rearrange("p (k c) -> p k c", c=4),
                        in0=rg[:].rearrange("p (k c) -> p k c", c=4),
                        in1=qr[:].rearrange("p (o c) -> p o c", o=1).to_broadcast(
                            [P, NCAND, 4]
                        ),
                        op=mybir.AluOpType.mult,
                    )
                    sex = sm.tile([P, NCAND], F32, tag="sex")
                    nc.vector.tensor_reduce(
                        out=sex[:],
                        in_=prod[:].rearrange("p (k c) -> p k c", c=4),
                        axis=mybir.AxisListType.X,
                        op=mybir.AluOpType.add,
                    )
                    smax = sm.tile([P, 1], F32, tag="smax")
                    nc.vector.tensor_reduce(
                        out=smax[:], in_=sex[:], axis=mybir.AxisListType.X,
                        op=mybir.AluOpType.max,
                    )
                    pen = sm.tile([P, NCAND], F32, tag="pen")
                    # pen = 0 where sex == smax else 16384
                    nc.vector.tensor_scalar(
                        out=pen[:], in0=sex[:], scalar1=smax[:, 0:1],
                        scalar2=None, op0=mybir.AluOpType.is_ge,
                    )
                    nc.vector.tensor_scalar(
                        out=pen[:], in0=pen[:], scalar1=-16384.0, scalar2=16384.0,
                        op0=mybir.AluOpType.mult, op1=mybir.AluOpType.add,
                    )
                    nc.vector.tensor_add(pen[:], pen[:], canf[:])
                    jst = sm.tile([P, 1], F32, tag="jst")
                    nc.vector.tensor_reduce(
                        out=jst[:], in_=pen[:], axis=mybir.AxisListType.X,
                        op=mybir.AluOpType.min,
                    )
                    jsti = sm.tile([P, 1], I32, tag="jsti")
                    nc.vector.tensor_copy(jsti[:], jst[:])
                    nc.vector.tensor_copy(
                        idxbuf[:, d * n_blocks + b : d * n_blocks + b + 1], jsti[:]
                    )

                    # MSE contribution: gather e_ref rows, squared diff vs e_query
                    eg = sm.tile([P, 4], F32, tag="eg")
                    nc.gpsimd.indirect_dma_start(
                        out=eg[:],
                        out_offset=None,
                        in_=erow_d[d][:],
                        in_offset=bass.IndirectOffsetOnAxis(ap=jsti[:, 0:1], axis=0),
                    )
                    eqt = sm.tile([P, 4], F32, tag="eqt")
                    nc.sync.dma_start(
                        out=eqt[:], in_=eq[d * QPC + b * P : d * QPC + (b + 1) * P, :]
                    )
                    diff = sm.tile([P, 4], F32, tag="diff")
                    nc.vector.tensor_sub(diff[:], eqt[:], eg[:])
                    sq = sm.tile([P, 4], F32, tag="sq")
                    nc.vector.tensor_mul(sq[:], diff[:], diff[:])
                    nc.vector.tensor_reduce(
                        out=lossparts[:, d * n_blocks + b : d * n_blocks + b + 1],
                        in_=sq[:],
                        axis=mybir.AxisListType.X,
                        op=mybir.AluOpType.add,
                    )

            lsum = accp.tile([P, 2], F32)
            nc.vector.tensor_reduce(
                out=lsum[:],
                in_=lossparts[:].rearrange("p (d b) -> p d b", d=2),
                axis=mybir.AxisListType.X,
                op=mybir.AluOpType.add,
            )
            nc.sync.dma_start(out=loss_out[:], in_=lsum[:])
            nc.sync.dma_start(out=idx_out[:], in_=idxbuf[:])
    nc.compile()
    return nc


def _prep_inputs(pred_vertices, trg_vertices, pred_e, trg_e):
    """Host-side layout prep (transposes + norm augmentation) and 8-way shard."""
    pv = np.ascontiguousarray(pred_vertices[0])  # [V,3]
    tv = np.ascontiguousarray(trg_vertices[0])
    pe = np.ascontiguousarray(pred_e[0])
    te = np.ascontiguousarray(trg_e[0])

    def aug_ref_T(r):  # [5, V]: x, y, z, -|r|^2/2, 1
        n2 = ((r * r).sum(1) * np.float32(0.5)).astype(np.float32)
        return np.concatenate(
            [r.T, -n2[None, :], np.ones((1, V), np.float32)], axis=0
        )

    def aug_q_T(q):  # [5, Vq]: x, y, z, 1, -|q|^2/2
        n2 = ((q * q).sum(1) * np.float32(0.5)).astype(np.float32)
        return np.concatenate(
            [q.T, np.ones((1, q.shape[0]), np.float32), -n2[None, :]], axis=0
        )

    def row4(r):  # [V, 4]: x, y, z, -|r|^2/2
        n2 = ((r * r).sum(1) * np.float32(0.5)).astype(np.float32)
        return np.ascontiguousarray(
            np.concatenate([r, -n2[:, None]], axis=1).astype(np.float32)
        )

    def pad4(e):  # [*, 4] with zero 4th col
        return np.ascontiguousarray(
            np.concatenate([e, np.zeros((e.shape[0], 1), np.float32)], axis=1)
        )

    # dir A: queries tv vs refs pv; dir B: queries pv vs refs tv
    rT = np.ascontiguousarray(
        np.concatenate([aug_ref_T(pv), aug_ref_T(tv)], axis=1)
    )  # [5, 2V]
    rrowA, rrowB = row4(pv), row4(tv)
    erowA, erowB = pad4(pe), pad4(te)
    qT_A, qT_B = aug_q_T(tv), aug_q_T(pv)  # [5, V] each
    qrow_A = pad4(tv)
    qrow_A[:, 3] = 1.0
    qrow_B = pad4(pv)
    qrow_B[:, 3] = 1.0
    eq_A, eq_B = pad4(te), pad4(pe)

    in_maps = []
    for c in range(NCORES):
        sl = slice(c * QPC, (c + 1) * QPC)
        in_maps.append(
            {
                "qT": np.ascontiguousarray(
                    np.concatenate([qT_A[:, sl], qT_B[:, sl]], axis=1)
                ),
                "rT": rT,
                "rrowA": rrowA,
                "rrowB": rrowB,
                "erowA": erowA,
                "erowB": erowB,
                "qrow": np.ascontiguousarray(
                    np.concatenate([qrow_A[sl], qrow_B[sl]], axis=0)
                ),
                "eq": np.ascontiguousarray(
                    np.concatenate([eq_A[sl], eq_B[sl]], axis=0)
                ),
            }
        )
    return in_maps


def run_device(in_maps):
    if "nc" not in _CACHE:
        _CACHE["nc"] = build()
    return run_bass_kernel_spmd(_CACHE["nc"], in_maps, list(range(NCORES))).results


def kernel(pred_vertices, trg_vertices, pred_e, trg_e):
    in_maps = _prep_inputs(pred_vertices, trg_vertices, pred_e, trg_e)
    results = run_device(in_maps)
    total = np.float64(0.0)
    for d in range(2):
        s = np.float64(0.0)
        for c in range(NCORES):
            s += results[c]["loss_out"][:, d].astype(np.float64).sum()
        total += s / np.float64(V * 3)
    return np.float32(total)


def kernel_indices(pred_vertices, trg_vertices, pred_e, trg_e):
    """Debug helper: returns (idxA [V], idxB [V]) nearest-neighbor indices."""
    in_maps = _prep_inputs(pred_vertices, trg_vertices, pred_e, trg_e)
    results = run_device(in_maps)
    idxA = np.zeros(V, np.int64)
    idxB = np.zeros(V, np.int64)
    for c in range(NCORES):
        io = results[c]["idx_out"]  # [P, 2*NBLK]
        for b in range(NBLK):
            rows = slice(c * QPC + b * P, c * QPC + (b + 1) * P)
            idxA[rows] = io[:, b]
            idxB[rows] = io[:, NBLK + b]
    return idxA, idxB


# revision 7
# speedup vs baseline: 4.3527x; 4.3527x over previous
"""Chamfer loss (two 16384x16384 1-NN searches + gathered MSE) on 8 Trainium2 cores.

Strategy (per core, queries sharded 8-way):
  - PE: score matrix S[i,j] = q_i . r_j - |r_j|^2/2 - |q_i|^2/2 = -d(i,j)/2 via
    K=5 augmented fp32r matmuls ([5,128] x [5,512] -> PSUM), 32 j-tiles per
    128-query block. argmax_j S = argmin_j dist.
  - DVE: pairwise-max fold of PSUM quads into fp16 (Spair 8192 -> Squad 4096),
    then max/max_index top-8 slots -> 32 original candidate indices per query.
  - Exact refinement: indirect-DMA gather of the 32 candidate rows (fp32
    coords + norm), recompute exact fp32 scores, first-index argmax ->
    exact fp32 1-NN index (immune to fp32r/fp16 approximation noise).
  - MSE: indirect-DMA gather of ref-side e rows by final index, squared-diff
    accumulated per partition; host sums 128x2x8 partials and scales.

The kernel is self-contained: hardcoded shapes for V=16384, 8 cores.
"""
import sys

sys.path.insert(0, "/opt/trn_rl_repo")

import numpy as np

import concourse.bass as bass
import concourse.bacc as bacc
import concourse.mybir as mybir
from concourse.tile import TileContext
from concourse.bass_utils import run_bass_kernel_spmd

P = 128          # partitions / queries per block
V = 16384        # reference points
NCORES = 8
QPC = V // NCORES            # queries per core per direction (2048)
NBLK = QPC // P              # query blocks per direction (16)
NT = V // 512                # j-tiles of 512 (32)
NQUAD = NT // 4              # psum quads of 4 tiles (8)
NCAND = 32                   # candidates per query after top-8 quad slots
ACT_QUADS = 8                # PSUM->SBUF copy quads handled by ScalarE (rest: DVE)
F16 = mybir.dt.float16
F32 = mybir.dt.float32
F32R = mybir.dt.float32r
I32 = mybir.dt.int32
U32 = mybir.dt.uint32

_CACHE = {}


def build(n_blocks=NBLK):
    nc = bacc.Bacc()
    # per-core inputs
    qT = nc.dram_tensor("qT", [5, 2 * QPC], F32, kind="ExternalInput")
    rT = nc.dram_tensor("rT", [5, 2 * V], F32, kind="ExternalInput")
    rrowA = nc.dram_tensor("rrowA", [V, 4], F32, kind="ExternalInput")
    rrowB = nc.dram_tensor("rrowB", [V, 4], F32, kind="ExternalInput")
    erowA = nc.dram_tensor("erowA", [V, 4], F32, kind="ExternalInput")
    erowB = nc.dram_tensor("erowB", [V, 4], F32, kind="ExternalInput")
    qrow = nc.dram_tensor("qrow", [2 * QPC, 4], F32, kind="ExternalInput")
    eq = nc.dram_tensor("eq", [2 * QPC, 4], F32, kind="ExternalInput")
    loss_out = nc.dram_tensor("loss_out", [P, 2], F32, kind="ExternalOutput")
    idx_out = nc.dram_tensor("idx_out", [P, 2 * n_blocks], I32, kind="ExternalOutput")

    rrow_d = [rrowA, rrowB]
    erow_d = [erowA, erowB]

    with TileContext(nc) as tc:
        with (
            tc.tile_pool(name="tab", bufs=1) as tab,
            tc.tile_pool(name="sb", bufs=2) as sb,
            tc.tile_pool(name="fold", bufs=1) as fold,
            tc.tile_pool(name="small", bufs=3) as sm,
            tc.tile_pool(name="acc", bufs=1) as accp,
            tc.tile_pool(name="ps", bufs=2, space="PSUM") as ps,
        ):
            qt = tab.tile([5, 2 * QPC], F32R)
            nc.sync.dma_start(out=qt[:], in_=qT[:].bitcast(F32R))
            lossparts = accp.tile([P, 2 * n_blocks], F32)
            nc.vector.memset(lossparts[:], 0.0)
            idxbuf = accp.tile([P, 2 * n_blocks], I32)

            for d in range(2):
                rt = tab.tile([5, V], F32R, tag="rt")
                nc.sync.dma_start(
                    out=rt[:], in_=rT[:, d * V : (d + 1) * V].bitcast(F32R)
                )
                for b in range(n_blocks):
                    lhsT = qt[:, d * QPC + b * P : d * QPC + (b + 1) * P]
                    sfull = sb.tile([P, V], F16, tag="sfull")
                    for g in range(NQUAD):
                        psq = ps.tile([P, 2048], F32, tag="psq")
                        for t in range(4):
                            nc.tensor.matmul(
                                out=psq[:, t * 512 : (t + 1) * 512],
                                lhsT=lhsT,
                                rhs=rt[:, (g * 4 + t) * 512 : (g * 4 + t + 1) * 512],
                                start=True,
                                stop=True,
                            )
                        dst = sfull[:, g * 2048 : (g + 1) * 2048]
                        if g < ACT_QUADS:
                            nc.scalar.copy(dst, psq[:])
                        else:
                            nc.vector.tensor_copy(dst, psq[:])
                    spair = fold.tile([P, 8192], F16, tag="spair")
                    nc.vector.tensor_tensor(
                        out=spair[:],
                        in0=sfull[:, 0:8192],
                        in1=sfull[:, 8192:V],
                        op=mybir.AluOpType.max,
                    )
                    squad = fold.tile([P, 4096], F16, tag="squad")
                    nc.vector.tensor_tensor(
                        out=squad[:],
                        in0=spair[:, 0:4096],
                        in1=spair[:, 4096:8192],
                        op=mybir.AluOpType.max,
                    )
                    m8 = sm.tile([P, 8], F16, tag="m8")
                    i8 = sm.tile([P, 8], U32, tag="i8")
                    nc.vector.max(out=m8[:], in_=squad[:])
                    nc.vector.max_index(out=i8[:], in_max=m8[:], in_values=squad[:])

                    # slot v -> candidates v + {0, 4096, 8192, 12288}
                    i8f = sm.tile([P, 8], F32, tag="i8f")
                    nc.vector.tensor_copy(i8f[:], i8[:])
                    canf = sm.tile([P, NCAND], F32, tag="canf")
                    for o, off in enumerate([0.0, 4096.0, 8192.0, 12288.0]):
                        nc.vector.tensor_scalar(
                            out=canf[:, o * 8 : (o + 1) * 8], in0=i8f[:],
                            scalar1=off, scalar2=None, op0=mybir.AluOpType.add,
                        )
                    cani = sm.tile([P, NCAND], I32, tag="cani")
                    nc.vector.tensor_copy(cani[:], canf[:])

                    # gather candidate rows (x, y, z, -|r|^2/2) and refine exactly
                    rg = sm.tile([P, NCAND * 4], F32, tag="rg")
                    for k in range(NCAND):
                        nc.gpsimd.indirect_dma_start(
                            out=rg[:, k * 4 : (k + 1) * 4],
                            out_offset=None,
                            in_=rrow_d[d][:],
                            in_offset=bass.IndirectOffsetOnAxis(
                                ap=cani[:, k : k + 1], axis=0
                            ),
                        )
                    qr = sm.tile([P, 4], F32, tag="qr")
                    nc.sync.dma_start(
                        out=qr[:], in_=qrow[d * QPC + b * P : d * QPC + (b + 1) * P, :]
                    )
                    prod = sm.tile([P, NCAND * 4], F32, tag="prod")
                    nc.vector.tensor_tensor(
                        out=prod[:].rearrange("p (k c) -> p k c", c=4),
                        in0=rg[:].rearrange("p (k c) -> p k c", c=4),
                        in1=qr[:].rearrange("p (o c) -> p o c", o=1).to_broadcast(
                            [P, NCAND, 4]
                        ),
                        op=mybir.AluOpType.mult,
                    )
                    sex = sm.tile([P, NCAND], F32, tag="sex")
                    nc.vector.tensor_reduce(
                        out=sex[:],
                        in_=prod[:].rearrange("p (k c) -> p k c", c=4),
                        axis=mybir.AxisListType.X,
                        op=mybir.AluOpType.add,
                    )
                    smax = sm.tile([P, 1], F32, tag="smax")
                    nc.vector.tensor_reduce(
                        out=smax[:], in_=sex[:], axis=mybir.AxisListType.X,
                        op=mybir.AluOpType.max,
                    )
                    pen = sm.tile([P, NCAND], F32, tag="pen")
                    # pen = 0 where sex == smax else 16384
                    nc.vector.tensor_scalar(
                        out=pen[:], in0=sex[:], scalar1=smax[:, 0:1],
                        scalar2=None, op0=mybir.AluOpType.is_ge,
                    )
                    nc.vector.tensor_scalar(
                        out=pen[:], in0=pen[:], scalar1=-16384.0, scalar2=16384.0,
                        op0=mybir.AluOpType.mult, op1=mybir.AluOpType.add,
                    )
                    nc.vector.tensor_add(pen[:], pen[:], canf[:])
                    jst = sm.tile([P, 1], F32, tag="jst")
                    nc.vector.tensor_reduce(
                        out=jst[:], in_=pen[:], axis=mybir.AxisListType.X,
                        op=mybir.AluOpType.min,
                    )
                    jsti = sm.tile([P, 1], I32, tag="jsti")
                    nc.vector.tensor_copy(jsti[:], jst[:])
                    nc.vector.tensor_copy(
                        idxbuf[:, d * n_blocks + b : d * n_blocks + b + 1], jsti[:]
                    )

                    # MSE contribution: gather e_ref rows, squared diff vs e_query
                    eg = sm.tile([P, 4], F32, tag="eg")
                    nc.gpsimd.indirect_dma_start(
                        out=eg[:],
                        out_offset=None,
                        in_=erow_d[d][:],
                        in_offset=bass.IndirectOffsetOnAxis(ap=jsti[:, 0:1], axis=0),
                    )
                    eqt = sm.tile([P, 4], F32, tag="eqt")
                    nc.sync.dma_start(
                        out=eqt[:], in_=eq[d * QPC + b * P : d * QPC + (b + 1) * P, :]
                    )
                    diff = sm.tile([P, 4], F32, tag="diff")
                    nc.vector.tensor_sub(diff[:], eqt[:], eg[:])
                    sq = sm.tile([P, 4], F32, tag="sq")
                    nc.vector.tensor_mul(sq[:], diff[:], diff[:])
                    nc.vector.tensor_reduce(
                        out=lossparts[:, d * n_blocks + b : d * n_blocks + b + 1],
                        in_=sq[:],
                        axis=mybir.AxisListType.X,
                        op=mybir.AluOpType.add,
                    )

            lsum = accp.tile([P, 2], F32)
            nc.vector.tensor_reduce(
                out=lsum[:],
                in_=lossparts[:].rearrange("p (d b) -> p d b", d=2),
                axis=mybir.AxisListType.X,
                op=mybir.AluOpType.add,
            )
            nc.sync.dma_start(out=loss_out[:], in_=lsum[:])
            nc.sync.dma_start(out=idx_out[:], in_=idxbuf[:])
    nc.compile()
    return nc


def _prep_inputs(pred_vertices, trg_vertices, pred_e, trg_e):
    """Host-side layout prep (transposes + norm augmentation) and 8-way shard."""
    pv = np.ascontiguousarray(pred_vertices[0])  # [V,3]
    tv = np.ascontiguousarray(trg_vertices[0])
    pe = np.ascontiguousarray(pred_e[0])
    te = np.ascontiguousarray(trg_e[0])

    def aug_ref_T(r):  # [5, V]: x, y, z, -|r|^2/2, 1
        n2 = ((r * r).sum(1) * np.float32(0.5)).astype(np.float32)
        return np.concatenate(
            [r.T, -n2[None, :], np.ones((1, V), np.float32)], axis=0
        )

    def aug_q_T(q):  # [5, Vq]: x, y, z, 1, -|q|^2/2
        n2 = ((q * q).sum(1) * np.float32(0.5)).astype(np.float32)
        return np.concatenate(
            [q.T, np.ones((1, q.shape[0]), np.float32), -n2[None, :]], axis=0
        )

    def row4(r):  # [V, 4]: x, y, z, -|r|^2/2
        n2 = ((r * r).sum(1) * np.float32(0.5)).astype(np.float32)
        return np.ascontiguousarray(
            np.concatenate([r, -n2[:, None]], axis=1).astype(np.float32)
        )

    def pad4(e):  # [*, 4] with zero 4th col
        return np.ascontiguousarray(
            np.concatenate([e, np.zeros((e.shape[0], 1), np.float32)], axis=1)
        )

    # dir A: queries tv vs refs pv; dir B: queries pv vs refs tv
    rT = np.ascontiguousarray(
        np.concatenate([aug_ref_T(pv), aug_ref_T(tv)], axis=1)
    )  # [5, 2V]
    rrowA, rrowB = row4(pv), row4(tv)
    erowA, erowB = pad4(pe), pad4(te)
    qT_A, qT_B = aug_q_T(tv), aug_q_T(pv)  # [5, V] each
    qrow_A = pad4(tv)
    qrow_A[:, 3] = 1.0
    qrow_B = pad4(pv)
    qrow_B[:, 3] = 1.0
    eq_A, eq_B = pad4(te), pad4(pe)

    in_maps = []
    for c in range(NCORES):
        sl = slice(c * QPC, (c + 1) * QPC)
        in_maps.append(
            {
                "qT": np.ascontiguousarray(
                    np.concatenate([qT_A[:, sl], qT_B[:, sl]], axis=1)
                ),
                "rT": rT,
                "rrowA": rrowA,
                "rrowB": rrowB,
                "erowA": erowA,
                "erowB": erowB,
                "qrow": np.ascontiguousarray(
                    np.concatenate([qrow_A[sl], qrow_B[sl]], axis=0)
                ),
                "eq": np.ascontiguousarray(
                    np.concatenate([eq_A[sl], eq_B[sl]], axis=0)
                ),
            }
        )
    return in_maps


def run_device(in_maps):
    if "nc" not in _CACHE:
        _CACHE["nc"] = build()
    return run_bass_kernel_spmd(_CACHE["nc"], in_maps, list(range(NCORES))).results


def kernel(pred_vertices, trg_vertices, pred_e, trg_e):
    in_maps = _prep_inputs(pred_vertices, trg_vertices, pred_e, trg_e)
    results = run_device(in_maps)
    total = np.float64(0.0)
    for d in range(2):
        s = np.float64(0.0)
        for c in range(NCORES):
            s += results[c]["loss_out"][:, d].astype(np.float64).sum()
        total += s / np.float64(V * 3)
    return np.float32(total)


def kernel_indices(pred_vertices, trg_vertices, pred_e, trg_e):
    """Debug helper: returns (idxA [V], idxB [V]) nearest-neighbor indices."""
    in_maps = _prep_inputs(pred_vertices, trg_vertices, pred_e, trg_e)
    results = run_device(in_maps)
    idxA = np.zeros(V, np.int64)
    idxB = np.zeros(V, np.int64)
    for c in range(NCORES):
        io = results[c]["idx_out"]  # [P, 2*NBLK]
        for b in range(NBLK):
            rows = slice(c * QPC + b * P, c * QPC + (b + 1) * P)
            idxA[rows] = io[:, b]
            idxB[rows] = io[:, NBLK + b]
    return idxA, idxB


# revision 8
# speedup vs baseline: 4.9726x; 1.1424x over previous
"""Chamfer loss (two 16384x16384 1-NN searches + gathered MSE) on 8 Trainium2 cores.

Device (per core, queries sharded 8-way across cores):
  - PE: score matrix S[i,j] = q_i . r_j - |r_j|^2/2 - |q_i|^2/2 = -d(i,j)/2 via
    K=5 augmented fp32r matmuls ([5,128] x [5,512] -> PSUM), 32 j-tiles per
    128-query block. argmax_j S = argmin_j dist. This is >99.7% of the FLOPs.
  - ScalarE drains PSUM quads to an fp16 score row (Sfull [128, 16384]).
  - DVE folds Sfull twice by elementwise max (8192 -> 4096 slots; slot v covers
    candidates v + {0, 4096, 8192, 12288}), then max/max_index extract the
    top-8 fp16 slots per query -> 32 candidate indices covering the true
    argmin with a wide margin over fp32r/fp16 rounding noise (worst-case
    crowding at that noise window is ~23 candidates, measured offline).

Host:
  - Exact fp32 re-scoring of the 32 candidates per query (the same numpy-fp32
    formula as the reference), first-index argmax -> exact 1-NN index.
  - Gather e rows, squared-error means in f64 -> final f32 scalar.

Per-row indirect-DMA gathers on TRN2 cost ~1us/row of descriptor generation
(measured ~100ms for the on-device refinement variant), so the tiny
refinement lives on the host instead.
"""
import sys

sys.path.insert(0, "/opt/trn_rl_repo")

import numpy as np

import concourse.bass as bass
import concourse.bacc as bacc
import concourse.mybir as mybir
from concourse.tile import TileContext
from concourse.bass_utils import run_bass_kernel_spmd

P = 128          # partitions / queries per block
V = 16384        # reference points
NCORES = 8
QPC = V // NCORES            # queries per core per direction (2048)
NBLK = QPC // P              # query blocks per direction (16)
NQUAD = 8                    # psum quads of 4 j-tiles (4 x 512 cols)
NCAND = 32                   # candidates per query: top-8 slots x 4
ACT_QUADS = 8                # PSUM->SBUF drain quads on ScalarE (rest: DVE)
F16 = mybir.dt.float16
F32 = mybir.dt.float32
F32R = mybir.dt.float32r
U32 = mybir.dt.uint32

_CACHE = {}


def build(n_blocks=NBLK):
    nc = bacc.Bacc()
    qT = nc.dram_tensor("qT", [5, 2 * QPC], F32, kind="ExternalInput")
    rT = nc.dram_tensor("rT", [5, 2 * V], F32, kind="ExternalInput")
    slot_out = nc.dram_tensor(
        "slot_out", [P, 2 * n_blocks * 8], U32, kind="ExternalOutput"
    )

    with TileContext(nc) as tc:
        with (
            tc.tile_pool(name="tab", bufs=1) as tab,
            tc.tile_pool(name="sb", bufs=2) as sb,
            tc.tile_pool(name="fold", bufs=1) as fold,
            tc.tile_pool(name="small", bufs=3) as sm,
            tc.tile_pool(name="acc", bufs=1) as accp,
            tc.tile_pool(name="ps", bufs=2, space="PSUM") as ps,
        ):
            qt = tab.tile([5, 2 * QPC], F32R)
            nc.sync.dma_start(out=qt[:], in_=qT[:].bitcast(F32R))
            slotbuf = accp.tile([P, 2 * n_blocks * 8], U32)

            for d in range(2):
                rt = tab.tile([5, V], F32R, tag="rt")
                nc.sync.dma_start(
                    out=rt[:], in_=rT[:, d * V : (d + 1) * V].bitcast(F32R)
                )
                for b in range(n_blocks):
                    lhsT = qt[:, d * QPC + b * P : d * QPC + (b + 1) * P]
                    sfull = sb.tile([P, V], F16, tag="sfull")
                    for g in range(NQUAD):
                        psq = ps.tile([P, 2048], F32, tag="psq")
                        for t in range(4):
                            nc.tensor.matmul(
                                out=psq[:, t * 512 : (t + 1) * 512],
                                lhsT=lhsT,
                                rhs=rt[:, (g * 4 + t) * 512 : (g * 4 + t + 1) * 512],
                                start=True,
                                stop=True,
                            )
                        dst = sfull[:, g * 2048 : (g + 1) * 2048]
                        if g < ACT_QUADS:
                            nc.scalar.copy(dst, psq[:])
                        else:
                            nc.vector.tensor_copy(dst, psq[:])
                    spair = fold.tile([P, 8192], F16, tag="spair")
                    nc.vector.tensor_tensor(
                        out=spair[:],
                        in0=sfull[:, 0:8192],
                        in1=sfull[:, 8192:V],
                        op=mybir.AluOpType.max,
                    )
                    squad = fold.tile([P, 4096], F16, tag="squad")
                    nc.vector.tensor_tensor(
                        out=squad[:],
                        in0=spair[:, 0:4096],
                        in1=spair[:, 4096:8192],
                        op=mybir.AluOpType.max,
                    )
                    m8 = sm.tile([P, 8], F16, tag="m8")
                    col = (d * n_blocks + b) * 8
                    nc.vector.max(out=m8[:], in_=squad[:])
                    nc.vector.max_index(
                        out=slotbuf[:, col : col + 8], in_max=m8[:], in_values=squad[:]
                    )
            nc.sync.dma_start(out=slot_out[:], in_=slotbuf[:])
    nc.compile()
    return nc


def _aug_tables(pred_vertices, trg_vertices):
    pv = np.ascontiguousarray(pred_vertices[0])  # [V,3]
    tv = np.ascontiguousarray(trg_vertices[0])

    def aug_ref_T(r):  # [5, V]: x, y, z, -|r|^2/2, 1
        n2 = ((r * r).sum(1) * np.float32(0.5)).astype(np.float32)
        return np.concatenate(
            [r.T, -n2[None, :], np.ones((1, V), np.float32)], axis=0
        )

    def aug_q_T(q):  # [5, Vq]: x, y, z, 1, -|q|^2/2
        n2 = ((q * q).sum(1) * np.float32(0.5)).astype(np.float32)
        return np.concatenate(
            [q.T, np.ones((1, q.shape[0]), np.float32), -n2[None, :]], axis=0
        )

    rT = np.ascontiguousarray(np.concatenate([aug_ref_T(pv), aug_ref_T(tv)], axis=1))
    qT_A, qT_B = aug_q_T(tv), aug_q_T(pv)
    return pv, tv, rT, qT_A, qT_B


def _prep_inputs(pred_vertices, trg_vertices, pred_e=None, trg_e=None):
    _, _, rT, qT_A, qT_B = _aug_tables(pred_vertices, trg_vertices)
    in_maps = []
    for c in range(NCORES):
        sl = slice(c * QPC, (c + 1) * QPC)
        in_maps.append(
            {
                "qT": np.ascontiguousarray(
                    np.concatenate([qT_A[:, sl], qT_B[:, sl]], axis=1)
                ),
                "rT": rT,
            }
        )
    return in_maps


def run_device(in_maps):
    if "nc" not in _CACHE:
        _CACHE["nc"] = build()
    return run_bass_kernel_spmd(_CACHE["nc"], in_maps, list(range(NCORES))).results


_OFFS = np.array([0, 4096, 8192, 12288], dtype=np.int64)


def _exact_indices(results, pv, tv):
    """Top-8 fp16 slots -> 32 candidates -> exact fp32 first-index argmax."""
    out = []
    for d, (q, r) in enumerate([(tv, pv), (pv, tv)]):
        slots = np.empty((V, 8), np.int64)
        for c in range(NCORES):
            so = results[c]["slot_out"]  # [P, 2*NBLK*8]
            for b in range(NBLK):
                rows = slice(c * QPC + b * P, c * QPC + (b + 1) * P)
                slots[rows] = so[:, (d * NBLK + b) * 8 : (d * NBLK + b + 1) * 8]
        cand = (slots[:, :, None] + _OFFS[None, None, :]).reshape(V, NCAND)
        n2 = ((r * r).sum(1) * np.float32(0.5)).astype(np.float32)
        rc = r[cand]                       # [V, 32, 3]
        s = np.einsum("vkc,vc->vk", rc, q).astype(np.float32) - n2[cand]
        smax = s.max(axis=1)
        masked = np.where(s >= smax[:, None], cand, 1 << 30)
        out.append(masked.min(axis=1))
    return out  # [idxA, idxB]


def kernel(pred_vertices, trg_vertices, pred_e, trg_e):
    pv, tv, _, _, _ = _aug_tables(pred_vertices, trg_vertices)
    in_maps = _prep_inputs(pred_vertices, trg_vertices)
    results = run_device(in_maps)
    idxA, idxB = _exact_indices(results, pv, tv)
    pe = np.ascontiguousarray(pred_e[0])
    te = np.ascontiguousarray(trg_e[0])
    lossA = ((te.astype(np.float64) - pe[idxA].astype(np.float64)) ** 2).sum() / (
        V * 3
    )
    lossB = ((pe.astype(np.float64) - te[idxB].astype(np.float64)) ** 2).sum() / (
        V * 3
    )
    return np.float32(lossA + lossB)


def kernel_indices(pred_vertices, trg_vertices, pred_e=None, trg_e=None):
    pv, tv, _, _, _ = _aug_tables(pred_vertices, trg_vertices)
    in_maps = _prep_inputs(pred_vertices, trg_vertices)
    results = run_device(in_maps)
    return _exact_indices(results, pv, tv)
